# revision 38
# baseline (speedup 1.0000x reference)
"""Trainium2 Bass kernel for nn_AttentionCell (Bahdanau attention + LSTM cell step).

Data-parallel over batch across 8 NeuronCores: each core gets B/8 = 64 batch
rows (all weights replicated) and computes its slice of (h_new, c_new, alpha)
independently — no collectives.

Per-core pipeline (shard b = 64, BT = b*T = 16384, all matmuls bf16 / f32 acc):
  phase A: hp^T = Wh^T prev_h^T + bh                        (PE, tiny)
  phase B: proj^T tiles [128h, 512bt] = Wi^T @ batch_H^T    (PE)
           + hp bias on DVE, whole-tile tanh on ACT -> bf16
           e = Ws^T tanh-tiles (M=1 matmuls, software-pipelined ~2 j behind
           and emitted per-j in batches of 4 to amortize the PE's 128<->32
           tile-size reconfig)
  phase C: softmax over t, split into two b-halves emitted INSIDE phase B so
           the serial chain hides under the tail of B; the block-diagonal
           alpha operand is built with DVE 32x32 transposes + strided copies
  phase D: context[64, 512] = alpha_blockdiag^T @ batch_H (one 128-matmul
           PSUM accumulation); the first 64 matmuls are interleaved into
           phase B's tail so the PE never idles or goes HAM-cold
  phase E: z = [context|onehots|prev_h|1] @ [K;R;b] -> LSTM gates -> h, c
           (the context-independent contraction rows run first while the
           context transpose is still in flight)

batch_H is passed from the host in BOTH layouts (transposed for phase B,
natural for phase D), pre-cast to bf16 and pre-packed so every load is a
single contiguous 2D DMA.
"""

import numpy as np
import ml_dtypes

import concourse.bass as bass
import concourse.mybir as mybir
import concourse.tile as tile
from concourse import bacc
from concourse.bass_utils import run_bass_kernel_spmd
from concourse.tile_rust import add_dep_helper

bf16 = ml_dtypes.bfloat16
F32 = mybir.dt.float32
BF16 = mybir.dt.bfloat16
AF = mybir.ActivationFunctionType
ALU = mybir.AluOpType
AX = mybir.AxisListType

NCORES = 8
B, T, C, H, NCLS = 512, 256, 512, 512, 96
P = B // NCORES          # 64 batch rows per core
BT = P * T               # 16384 flattened (b, t) rows per core
KXP = 1152               # LSTM contraction rows (1121) padded to 9*128
NJ = BT // 512           # 32 bt-chunks of 512
NKT = BT // 128          # 128 bt-chunks of 128

_CACHED = None


def _build():
    nc = bacc.Bacc("TRN2", target_bir_lowering=False, debug=False,
                   num_devices=NCORES)

    bhT = nc.dram_tensor("bhT", [C, BT], BF16, kind="ExternalInput")
    bhN = nc.dram_tensor("bhN", [BT // 4, 4 * C], BF16, kind="ExternalInput")
    wi = nc.dram_tensor("wi", [128, 4 * H], BF16, kind="ExternalInput")
    whE = nc.dram_tensor("whE", [128, 5 * H], BF16, kind="ExternalInput")
    phTe = nc.dram_tensor("phTe", [128, 5 * P], BF16, kind="ExternalInput")
    ws = nc.dram_tensor("ws", [128, 4], BF16, kind="ExternalInput")
    xrt = nc.dram_tensor("xrt", [128, 5 * P], BF16, kind="ExternalInput")
    krE = nc.dram_tensor("krE", [KXP, 4 * H], BF16, kind="ExternalInput")
    pc = nc.dram_tensor("pc", [P, H], F32, kind="ExternalInput")

    e_tmp = nc.dram_tensor("e_tmp", [BT], F32)  # internal staging for reshape
    h_out = nc.dram_tensor("h_out", [P, H], F32, kind="ExternalOutput")
    c_out = nc.dram_tensor("c_out", [P, H], F32, kind="ExternalOutput")
    alpha_out = nc.dram_tensor("alpha_out", [P, T], F32, kind="ExternalOutput")

    with tile.TileContext(nc) as tc:
        with (
            tc.tile_pool(name="const", bufs=1) as cpool,
            tc.tile_pool(name="work", bufs=1) as wpool,
            tc.tile_pool(name="bht", bufs=3) as bht_pool,
            tc.tile_pool(name="tanh", bufs=8) as tanh_pool,
            tc.tile_pool(name="parg", bufs=3) as parg_pool,
            tc.tile_pool(name="bhn", bufs=13) as bhn_pool,
            tc.tile_pool(name="kr", bufs=18) as kr_pool,
        ):
            # ---- constants / weights into SBUF ----
            whE_sb = cpool.tile([128, 5 * H], BF16)
            nc.gpsimd.dma_start(whE_sb[:], whE[:])
            phTe_sb = cpool.tile([128, 5 * P], BF16)
            nc.gpsimd.dma_start(phTe_sb[:], phTe[:])
            wi_sb = cpool.tile([128, 4 * H], BF16)           # [p, (kc h)]
            nc.gpsimd.dma_start(wi_sb[:], wi[:])
            ws_sb = cpool.tile([128, 4], BF16)               # [p, kc]
            nc.gpsimd.dma_start(ws_sb[:], ws[:])

            # block-diagonal alpha operand, one tensor per b-half so ctx
            # half 0 doesn't falsely depend on half 1's writers
            bd0 = wpool.tile([128, 64 * P], BF16)
            nc.vector.memset(bd0[:], 0.0)
            bd1 = wpool.tile([128, 64 * P], BF16)
            nc.vector.memset(bd1[:], 0.0)
            bd_half = [bd0, bd1]

            # phase-D rhs tiles arrive on the SWDGE queue (4 kt rows per DMA);
            # the first few are issued up front, the rest are spread through
            # the j-loop so they don't starve the bhT loads early on
            bhn_t = []

            def issue_bhn(g):
                bt_g = bhn_pool.tile([128, 4 * C], BF16, tag="bhn",
                                     name=f"bhn_t{g}")
                nc.gpsimd.dma_start(bt_g[:], bhN[g * 128:(g + 1) * 128, :])
                bhn_t.append(bt_g)


            # ---- phase A: hp^T [128h x 4hc, 64b] (bh folded via ones row) ----
            hpT_sb = wpool.tile([128, 4 * P], F32)
            with tc.tile_pool(name="psA", bufs=2, space="PSUM") as psA:
                for hc in range(4):
                    hp_ps = psA.tile([128, P], F32)
                    for kk in range(5):
                        nc.tensor.matmul(
                            hp_ps[:],
                            whE_sb[:, kk * H + hc * 128:kk * H + (hc + 1) * 128],
                            phTe_sb[:, kk * P:(kk + 1) * P],
                            start=(kk == 0), stop=(kk == 4),
                        )
                    nc.vector.tensor_copy(hpT_sb[:, hc * P:(hc + 1) * P], hp_ps[:])

            # ---- phases B + C + D (fused) ----
            e_stage = wpool.tile([128, (NJ // 4) * 512], F32)
            e_bt = wpool.tile([P, T], F32)
            mx_neg = wpool.tile([P, 1], F32)
            ex = wpool.tile([P, T], F32)
            ssum = wpool.tile([P, 1], F32)
            rsum = wpool.tile([P, 1], F32)
            alpha_sb = wpool.tile([P, T], F32)
            at0 = wpool.tile([32, T], F32)
            at1 = wpool.tile([32, T], F32)
            atp = [at0, at1]
            ctx_bf = wpool.tile([P, H], BF16)
            xs_ctx = wpool.tile([128, 4 * P], BF16)

            def softmax_half(hi):
                # e rows for b in [32*hi, 32*hi+32) live in slots s=4*hi..4*hi+3
                lo = hi * (BT // 2)
                ew = nc.sync.dma_start(
                    e_tmp[lo:lo + BT // 2].rearrange("(s g n) -> g s n", s=4, g=4),
                    e_stage[0:128:32, hi * 2048:(hi + 1) * 2048].rearrange(
                        "g (s n) -> g s n", s=4),
                )
                sl = slice(32 * hi, 32 * (hi + 1))
                er = nc.sync.dma_start(
                    e_bt[sl, :],
                    e_tmp[lo:lo + BT // 2].rearrange("(b t) -> b t", b=32))
                add_dep_helper(er.ins, ew.ins, True, f"e_tmp half{hi} order")
                nc.vector.tensor_reduce(mx_neg[sl, :], e_bt[sl, :], AX.X,
                                        ALU.max, negate=True)
                nc.scalar.activation(ex[sl, :], e_bt[sl, :], AF.Exp,
                                     bias=mx_neg[sl, :])
                nc.vector.tensor_reduce(ssum[sl, :], ex[sl, :], AX.X, ALU.add)
                nc.vector.reciprocal(rsum[sl, :], ssum[sl, :])
                nc.vector.tensor_scalar_mul(alpha_sb[sl, :], ex[sl, :], rsum[sl, :])
                nc.sync.dma_start(alpha_out[sl, :], alpha_sb[sl, :])
                # block-diagonal build: DVE 32x32 block transpose, then one
                # strided copy per t-block.  bd[p, (2b+i)*64 + b] = alpha[b, t]
                # with t = i*128 + p  ->  free offset 129*b + 64*i, stride 129
                at = atp[hi]
                nc.vector.transpose(at[:], alpha_sb[sl, :])
                for jb in range(8):
                    ihalf = jb // 4
                    pbase = (jb % 4) * 32
                    off = 64 * ihalf + 32 * hi
                    nc.vector.tensor_copy(
                        bd_half[hi][pbase:pbase + 32,
                                    off:off + 129 * 31 + 1:129],
                        at[:, 32 * jb:32 * (jb + 1)],
                    )

            with (
                tc.tile_pool(name="psB", bufs=4, space="PSUM") as psB,
                tc.tile_pool(name="psE", bufs=2, space="PSUM") as psE,
                tc.tile_pool(name="psD", bufs=2, space="PSUM") as psD,
            ):
                ctx_haps = [psD.tile([P, H], F32, tag="ctx", name="ctx_ps0"),
                            psD.tile([P, H], F32, tag="ctx", name="ctx_ps1")]

                def ctx_half(hi):
                    # each half is a self-contained accumulation group in its
                    # own PSUM bank (a group split across other matmul groups
                    # wedges the exec unit); the two banks are summed on DVE
                    for kt in range(64 * hi, 64 * (hi + 1)):
                        g, i = kt // 4, kt % 4
                        nc.tensor.matmul(
                            ctx_haps[hi][:],
                            bd_half[hi][:, (kt % 64) * P:(kt % 64 + 1) * P],
                            bhn_t[g][:, i * C:(i + 1) * C],
                            start=(kt % 64 == 0), stop=(kt % 64 == 63),
                        )

                # e-matmuls are software-pipelined ~2 j behind their tanh
                # producers and emitted per-j in batches of 4
                pending = []
                last_proj = [None]

                def flush_e():
                    for _ in range(4):
                        e_ps_f, hc_f, tanh_f, j_f = pending.pop(0)
                        mm = nc.tensor.matmul(
                            e_ps_f[:],
                            ws_sb[:, hc_f:hc_f + 1],
                            tanh_f[:],
                            start=(hc_f == 0), stop=(hc_f == 3),
                        )
                        if last_proj[0] is not None:
                            add_dep_helper(mm.ins, last_proj[0].ins, False,
                                           "software-pipeline e-mm behind proj")
                    g, s = (j_f % 4) * 32, j_f // 4
                    nc.scalar.copy(
                        e_stage[g:g + 1, s * 512:(s + 1) * 512], e_ps_f[:])
                    if j_f == 15:
                        softmax_half(0)

                kr_tiles = {}
                Z_KK = [4, 5, 6, 7, 8, 0, 1, 2, 3]

                def issue_kr(nn, kk):
                    kr_t = kr_pool.tile([128, 512], BF16, tag="kr",
                                        name=f"kr_{nn}_{kk}")
                    nc.sync.dma_start(
                        kr_t[:],
                        krE[kk * 128:(kk + 1) * 128, nn * 512:(nn + 1) * 512])
                    kr_tiles[(nn, kk)] = kr_t

                bht_t = None
                for j in range(NJ):
                    if j == 28:
                        # first two gate-slices' weights (18 = pool depth, so
                        # the sync FIFO never blocks on a slot here)
                        for nn in range(2):
                            for kk in Z_KK:
                                issue_kr(nn, kk)
                    if 2 <= j < 32:
                        issue_bhn(j - 2)
                    if j == 31:
                        issue_bhn(30)
                        issue_bhn(31)
                    if j % 4 == 0:
                        bht_t = [bht_pool.tile([128, 4 * 512], BF16,
                                               tag=f"bht{kc}",
                                               name=f"bht_t{j}_{kc}")
                                 for kc in range(4)]
                        for kc in range(4):
                            if j == 0:
                                # split the first group, spread across both
                                # DMA queues, so early slices land ASAP
                                eng0 = nc.sync if kc < 2 else nc.gpsimd
                                eng0.dma_start(
                                    bht_t[kc][:, 0:512],
                                    bhT[kc * 128:(kc + 1) * 128, 0:512],
                                )
                                eng0.dma_start(
                                    bht_t[kc][:, 512:2048],
                                    bhT[kc * 128:(kc + 1) * 128, 512:2048],
                                )
                            elif j == 4:
                                eng4 = nc.sync if kc < 2 else nc.gpsimd
                                for j2 in range(2):
                                    eng4.dma_start(
                                        bht_t[kc][:, j2 * 1024:(j2 + 1) * 1024],
                                        bhT[kc * 128:(kc + 1) * 128,
                                            (4 + j2 * 2) * 512:(6 + j2 * 2) * 512],
                                    )
                            else:
                                nc.sync.dma_start(
                                    bht_t[kc][:],
                                    bhT[kc * 128:(kc + 1) * 128,
                                        j * 512:(j + 4) * 512],
                                )
                    jj = j % 4
                    e_ps = psE.tile([1, 512], F32)
                    for hc in range(4):
                        proj_ps = psB.tile([128, 512], F32)
                        for kc in range(4):
                            last_proj[0] = nc.tensor.matmul(
                                proj_ps[:],
                                wi_sb[:, kc * H + hc * 128:kc * H + (hc + 1) * 128],
                                bht_t[kc][:, jj * 512:(jj + 1) * 512],
                                start=(kc == 0), stop=(kc == 3),
                            )
                        # hp bias-add on DVE (PSUM -> SBUF f32 staging), then a
                        # single whole-tile tanh on ACT
                        parg_t = parg_pool.tile([128, 512], F32)
                        for half in range(2):
                            b_idx = 2 * j + half
                            nc.vector.tensor_scalar_add(
                                parg_t[:, half * 256:(half + 1) * 256],
                                proj_ps[:, half * 256:(half + 1) * 256],
                                hpT_sb[:, hc * P + b_idx:hc * P + b_idx + 1],
                            )
                        tanh_t = tanh_pool.tile([128, 512], BF16)
                        nc.scalar.activation(tanh_t[:], parg_t[:], AF.Tanh)
                        pending.append((e_ps, hc, tanh_t, j))
                        if len(pending) >= 8:
                            flush_e()
                while pending:
                    flush_e()
                softmax_half(1)
                for nn in range(2, 4):
                    for kk in Z_KK:
                        issue_kr(nn, kk)
                # bd half h only has nonzero columns for its own 32 b-rows, so
                # each ctx half-accumulation yields a disjoint row range: no
                # sum needed, and each half's transpose chain starts as soon
                # as its own matmuls finish (hiding under the other half / z)
                ctp = wpool.tile([P, H], BF16)
                for hi2 in range(2):
                    # ctx half 0's operands are ready the moment B drains
                    ctx_half(hi2)
                    rs2 = slice(32 * hi2, 32 * (hi2 + 1))
                    nc.vector.tensor_copy(ctx_bf[rs2, :], ctx_haps[hi2][rs2, :])
                    nc.vector.transpose(ctp[rs2, :], ctx_bf[rs2, :])
                    for v in range(16):
                        kk2 = v // 4
                        pbase = (v % 4) * 32
                        nc.vector.tensor_copy(
                            xs_ctx[pbase:pbase + 32,
                                   kk2 * P + 32 * hi2:kk2 * P + 32 * hi2 + 32],
                            ctp[rs2, 32 * v:32 * (v + 1)],
                        )

            # ---- phase E: LSTM cell ----
            # xstack split: the context-independent rows live in their own
            # tensor so the first z-matmuls don't wait on the ctx transpose
            xs_rest = wpool.tile([128, 5 * P], BF16)
            nc.sync.dma_start(xs_rest[:], xrt[:])

            gi = wpool.tile([P, H], F32)
            gf = wpool.tile([P, H], F32)
            gg = wpool.tile([P, H], F32)
            go = wpool.tile([P, H], F32)
            gate_specs = [(gi, AF.Sigmoid), (gf, AF.Sigmoid),
                          (gg, AF.Tanh), (go, AF.Sigmoid)]
            with tc.tile_pool(name="psZ", bufs=1, space="PSUM") as psZ:
                z_ps = psZ.tile([P, 4 * H], F32)
                # gate-slice-major: each slice is one CONTIGUOUS closed
                # accumulation group (context-independent rows first inside
                # it), and its gate activation overlaps the next slice
                for nn in range(4):
                    for kk in Z_KK:
                        xsl = (xs_ctx[:, kk * P:(kk + 1) * P] if kk < 4 else
                               xs_rest[:, (kk - 4) * P:(kk - 3) * P])
                        nc.tensor.matmul(
                            z_ps[:, nn * 512:(nn + 1) * 512],
                            xsl,
                            kr_tiles[(nn, kk)][:],
                            start=(kk == 4), stop=(kk == 3),
                        )
                    g_t, g_fn = gate_specs[nn]
                    nc.scalar.activation(
                        g_t[:], z_ps[:, nn * 512:(nn + 1) * 512], g_fn)

            pc_sb = wpool.tile([P, H], F32)
            nc.sync.dma_start(pc_sb[:], pc[:])
            t1 = wpool.tile([P, H], F32)
            t2 = wpool.tile([P, H], F32)
            c_sb = wpool.tile([P, H], F32)
            nc.vector.tensor_mul(t1[:], gf[:], pc_sb[:])
            nc.vector.tensor_mul(t2[:], gi[:], gg[:])
            nc.vector.tensor_add(c_sb[:], t1[:], t2[:])
            nc.sync.dma_start(c_out[:], c_sb[:])
            tc_sb = wpool.tile([P, H], F32)
            nc.scalar.activation(tc_sb[:], c_sb[:], AF.Tanh)
            h_sb = wpool.tile([P, H], F32)
            nc.vector.tensor_mul(h_sb[:], go[:], tc_sb[:])
            nc.sync.dma_start(h_out[:], h_sb[:])

    nc.compile()
    return nc


def _get_nc():
    global _CACHED
    if _CACHED is None:
        _CACHED = _build()
    return _CACHED


def _prep_in_maps(inputs):
    batch_H = np.asarray(inputs["batch_H"], np.float32)
    prev_h = np.asarray(inputs["prev_h"], np.float32)
    prev_c = np.asarray(inputs["prev_c"], np.float32)
    onehots = np.asarray(inputs["char_onehots"], np.float32)
    Wi = np.asarray(inputs["Wi"], np.float32)
    Wh = np.asarray(inputs["Wh"], np.float32)
    bh = np.asarray(inputs["bh"], np.float32)
    Ws = np.asarray(inputs["Ws"], np.float32)
    K = np.asarray(inputs["K"], np.float32)
    R = np.asarray(inputs["R"], np.float32)
    b = np.asarray(inputs["b"], np.float32)

    def pack(a, k):  # [(k p), n] -> [p, (k n)]
        n = a.shape[1]
        return np.ascontiguousarray(
            a.reshape(k, 128, n).transpose(1, 0, 2).reshape(128, k * n))

    wi_b = pack(Wi, 4).astype(bf16)
    ws_b = np.ascontiguousarray(Ws.reshape(4, 128).T).astype(bf16)
    whE = np.zeros((640, H), np.float32)
    whE[:H] = Wh
    whE[H] = bh
    whE_b = pack(whE, 5).astype(bf16)
    krE = np.zeros((KXP, 4 * H), np.float32)
    krE[:C + NCLS] = K
    krE[C + NCLS:C + NCLS + H] = R
    krE[C + NCLS + H] = b
    krE_b = krE.astype(bf16)

    in_maps = []
    for i in range(NCORES):
        sl = slice(i * P, (i + 1) * P)
        bh_s = batch_H[sl].reshape(BT, C)
        ph_s = prev_h[sl]                       # [64, 512]
        phTe = np.zeros((640, P), np.float32)
        phTe[:H] = ph_s.T
        phTe[H] = 1.0
        xrt = np.zeros((640, P), np.float32)
        xrt[:NCLS] = onehots[sl].T
        xrt[NCLS:NCLS + H] = ph_s.T
        xrt[NCLS + H] = 1.0
        bhn_p = np.ascontiguousarray(
            bh_s.reshape(32, 4, 128, C).transpose(0, 2, 1, 3).reshape(BT // 4, 4 * C))
        in_maps.append({
            "bhT": np.ascontiguousarray(bh_s.T).astype(bf16),
            "bhN": bhn_p.astype(bf16),
            "wi": wi_b,
            "whE": whE_b,
            "phTe": pack(phTe, 5).astype(bf16),
            "ws": ws_b,
            "xrt": pack(xrt, 5).astype(bf16),
            "krE": krE_b,
            "pc": prev_c[sl],
        })
    return in_maps


def run(inputs, **run_kwargs):
    nc = _get_nc()
    in_maps = _prep_in_maps(inputs)
    res = run_bass_kernel_spmd(nc, in_maps, core_ids=list(range(NCORES)),
                               **run_kwargs)
    h = np.concatenate([res.results[i]["h_out"] for i in range(NCORES)], 0)
    c = np.concatenate([res.results[i]["c_out"] for i in range(NCORES)], 0)
    alpha = np.concatenate([res.results[i]["alpha_out"] for i in range(NCORES)], 0)
    return (h, c, alpha.reshape(B, T, 1)), res


def kernel(**inputs):
    (h, c, alpha), _ = run(inputs)
    return (h, c, alpha)


# revision 39
# speedup vs baseline: 1.0464x; 1.0464x over previous
"""Trainium2 Bass kernel for nn_AttentionCell (Bahdanau attention + LSTM cell step).

Data-parallel over batch across 8 NeuronCores: each core gets B/8 = 64 batch
rows (all weights replicated) and computes its slice of (h_new, c_new, alpha)
independently — no collectives.

Per-core pipeline (shard b = 64, BT = b*T = 16384, all matmuls bf16 / f32 acc):
  phase A: hp^T = Wh^T prev_h^T + bh                        (PE, tiny)
  phase B: proj^T tiles [128h, 512bt] = Wi^T @ batch_H^T    (PE)
           + hp bias on DVE, whole-tile tanh on ACT -> bf16
           e = Ws^T tanh-tiles (M=1 matmuls, software-pipelined ~2 j behind
           and emitted per-j in batches of 4 to amortize the PE's 128<->32
           tile-size reconfig)
  phase C: softmax over t, split into two b-halves emitted INSIDE phase B so
           the serial chain hides under the tail of B; the block-diagonal
           alpha operand is built with DVE 32x32 transposes + strided copies
  phase D: context[64, 512] = alpha_blockdiag^T @ batch_H (one 128-matmul
           PSUM accumulation); the first 64 matmuls are interleaved into
           phase B's tail so the PE never idles or goes HAM-cold
  phase E: z = [context|onehots|prev_h|1] @ [K;R;b] -> LSTM gates -> h, c
           (the context-independent contraction rows run first while the
           context transpose is still in flight)

batch_H is passed from the host in BOTH layouts (transposed for phase B,
natural for phase D), pre-cast to bf16 and pre-packed so every load is a
single contiguous 2D DMA.
"""

import numpy as np
import ml_dtypes

import concourse.bass as bass
import concourse.mybir as mybir
import concourse.tile as tile
from concourse import bacc
from concourse.bass_utils import run_bass_kernel_spmd
from concourse.tile_rust import add_dep_helper

bf16 = ml_dtypes.bfloat16
F32 = mybir.dt.float32
BF16 = mybir.dt.bfloat16
AF = mybir.ActivationFunctionType
ALU = mybir.AluOpType
AX = mybir.AxisListType

NCORES = 8
B, T, C, H, NCLS = 512, 256, 512, 512, 96
P = B // NCORES          # 64 batch rows per core
BT = P * T               # 16384 flattened (b, t) rows per core
KXP = 1152               # LSTM contraction rows (1121) padded to 9*128
NJ = BT // 512           # 32 bt-chunks of 512
NKT = BT // 128          # 128 bt-chunks of 128

_CACHED = None


def _build():
    nc = bacc.Bacc("TRN2", target_bir_lowering=False, debug=False,
                   num_devices=NCORES)

    bhT = nc.dram_tensor("bhT", [C, BT], BF16, kind="ExternalInput")
    bhN = nc.dram_tensor("bhN", [BT // 4, 4 * C], BF16, kind="ExternalInput")
    wi = nc.dram_tensor("wi", [128, 4 * H], BF16, kind="ExternalInput")
    whE = nc.dram_tensor("whE", [128, 5 * H], BF16, kind="ExternalInput")
    phTe = nc.dram_tensor("phTe", [128, 5 * P], BF16, kind="ExternalInput")
    ws = nc.dram_tensor("ws", [128, 4], BF16, kind="ExternalInput")
    xrt = nc.dram_tensor("xrt", [128, 5 * P], BF16, kind="ExternalInput")
    krE = nc.dram_tensor("krE", [KXP, 4 * H], BF16, kind="ExternalInput")
    pc = nc.dram_tensor("pc", [P, H], F32, kind="ExternalInput")

    e_tmp = nc.dram_tensor("e_tmp", [BT], F32)  # internal staging for reshape
    h_out = nc.dram_tensor("h_out", [P, H], F32, kind="ExternalOutput")
    c_out = nc.dram_tensor("c_out", [P, H], F32, kind="ExternalOutput")
    alpha_out = nc.dram_tensor("alpha_out", [P, T], F32, kind="ExternalOutput")

    with tile.TileContext(nc) as tc:
        with (
            tc.tile_pool(name="const", bufs=1) as cpool,
            tc.tile_pool(name="work", bufs=1) as wpool,
            tc.tile_pool(name="bht", bufs=3) as bht_pool,
            tc.tile_pool(name="tanh", bufs=8) as tanh_pool,
            tc.tile_pool(name="parg", bufs=3) as parg_pool,
            tc.tile_pool(name="bhn", bufs=13) as bhn_pool,
            tc.tile_pool(name="kr", bufs=5) as kr_pool,
        ):
            # ---- constants / weights into SBUF ----
            whE_sb = cpool.tile([128, 5 * H], BF16)
            nc.gpsimd.dma_start(whE_sb[:], whE[:])
            phTe_sb = cpool.tile([128, 5 * P], BF16)
            nc.gpsimd.dma_start(phTe_sb[:], phTe[:])
            wi_sb = cpool.tile([128, 4 * H], BF16)           # [p, (kc h)]
            nc.gpsimd.dma_start(wi_sb[:], wi[:])
            ws_sb = cpool.tile([128, 4], BF16)               # [p, kc]
            nc.gpsimd.dma_start(ws_sb[:], ws[:])

            # block-diagonal alpha operand, one tensor per b-half so ctx
            # half 0 doesn't falsely depend on half 1's writers
            bd0 = wpool.tile([128, 64 * P], BF16)
            nc.vector.memset(bd0[:], 0.0)
            bd1 = wpool.tile([128, 64 * P], BF16)
            nc.vector.memset(bd1[:], 0.0)
            bd_half = [bd0, bd1]

            # phase-D rhs tiles arrive on the SWDGE queue (4 kt rows per DMA);
            # the first few are issued up front, the rest are spread through
            # the j-loop so they don't starve the bhT loads early on
            bhn_t = []

            def issue_bhn(g):
                bt_g = bhn_pool.tile([128, 4 * C], BF16, tag="bhn",
                                     name=f"bhn_t{g}")
                nc.gpsimd.dma_start(bt_g[:], bhN[g * 128:(g + 1) * 128, :])
                bhn_t.append(bt_g)


            # ---- phase A: hp^T [128h x 4hc, 64b] (bh folded via ones row) ----
            hpT_sb = wpool.tile([128, 4 * P], F32)
            with tc.tile_pool(name="psA", bufs=2, space="PSUM") as psA:
                for hc in range(4):
                    hp_ps = psA.tile([128, P], F32)
                    for kk in range(5):
                        nc.tensor.matmul(
                            hp_ps[:],
                            whE_sb[:, kk * H + hc * 128:kk * H + (hc + 1) * 128],
                            phTe_sb[:, kk * P:(kk + 1) * P],
                            start=(kk == 0), stop=(kk == 4),
                        )
                    nc.vector.tensor_copy(hpT_sb[:, hc * P:(hc + 1) * P], hp_ps[:])

            # ---- phases B + C + D (fused) ----
            e_stage = wpool.tile([128, (NJ // 4) * 512], F32)
            e_bt = wpool.tile([P, T], F32)
            mx_neg = wpool.tile([P, 1], F32)
            ex = wpool.tile([P, T], F32)
            ssum = wpool.tile([P, 1], F32)
            rsum = wpool.tile([P, 1], F32)
            alpha_sb = wpool.tile([P, T], F32)
            at0 = wpool.tile([32, T], F32)
            at1 = wpool.tile([32, T], F32)
            atp = [at0, at1]
            ctx_bf = wpool.tile([P, H], BF16)
            xs_ctx = wpool.tile([128, 4 * P], BF16)

            def softmax_half(hi):
                # e rows for b in [32*hi, 32*hi+32) live in slots s=4*hi..4*hi+3
                lo = hi * (BT // 2)
                ew = nc.sync.dma_start(
                    e_tmp[lo:lo + BT // 2].rearrange("(s g n) -> g s n", s=4, g=4),
                    e_stage[0:128:32, hi * 2048:(hi + 1) * 2048].rearrange(
                        "g (s n) -> g s n", s=4),
                )
                sl = slice(32 * hi, 32 * (hi + 1))
                er = nc.sync.dma_start(
                    e_bt[sl, :],
                    e_tmp[lo:lo + BT // 2].rearrange("(b t) -> b t", b=32))
                add_dep_helper(er.ins, ew.ins, True, f"e_tmp half{hi} order")
                nc.vector.tensor_reduce(mx_neg[sl, :], e_bt[sl, :], AX.X,
                                        ALU.max, negate=True)
                nc.scalar.activation(ex[sl, :], e_bt[sl, :], AF.Exp,
                                     bias=mx_neg[sl, :])
                nc.vector.tensor_reduce(ssum[sl, :], ex[sl, :], AX.X, ALU.add)
                nc.vector.reciprocal(rsum[sl, :], ssum[sl, :])
                nc.vector.tensor_scalar_mul(alpha_sb[sl, :], ex[sl, :], rsum[sl, :])
                nc.sync.dma_start(alpha_out[sl, :], alpha_sb[sl, :])
                # block-diagonal build: DVE 32x32 block transpose, then one
                # strided copy per t-block.  bd[p, (2b+i)*64 + b] = alpha[b, t]
                # with t = i*128 + p  ->  free offset 129*b + 64*i, stride 129
                at = atp[hi]
                nc.vector.transpose(at[:], alpha_sb[sl, :])
                for jb in range(8):
                    ihalf = jb // 4
                    pbase = (jb % 4) * 32
                    off = 64 * ihalf + 32 * hi
                    nc.vector.tensor_copy(
                        bd_half[hi][pbase:pbase + 32,
                                    off:off + 129 * 31 + 1:129],
                        at[:, 32 * jb:32 * (jb + 1)],
                    )

            with (
                tc.tile_pool(name="psB", bufs=4, space="PSUM") as psB,
                tc.tile_pool(name="psE", bufs=2, space="PSUM") as psE,
                tc.tile_pool(name="psD", bufs=2, space="PSUM") as psD,
            ):
                ctx_haps = [psD.tile([P, H], F32, tag="ctx", name="ctx_ps0"),
                            psD.tile([P, H], F32, tag="ctx", name="ctx_ps1")]

                def ctx_half(hi):
                    # each half is a self-contained accumulation group in its
                    # own PSUM bank (a group split across other matmul groups
                    # wedges the exec unit); the two banks are summed on DVE
                    for kt in range(64 * hi, 64 * (hi + 1)):
                        g, i = kt // 4, kt % 4
                        nc.tensor.matmul(
                            ctx_haps[hi][:],
                            bd_half[hi][:, (kt % 64) * P:(kt % 64 + 1) * P],
                            bhn_t[g][:, i * C:(i + 1) * C],
                            start=(kt % 64 == 0), stop=(kt % 64 == 63),
                        )

                # e-matmuls are software-pipelined ~2 j behind their tanh
                # producers and emitted per-j in batches of 4
                pending = []
                last_proj = [None]

                def flush_e():
                    for _ in range(4):
                        e_ps_f, hc_f, tanh_f, j_f = pending.pop(0)
                        mm = nc.tensor.matmul(
                            e_ps_f[:],
                            ws_sb[:, hc_f:hc_f + 1],
                            tanh_f[:],
                            start=(hc_f == 0), stop=(hc_f == 3),
                        )
                        if last_proj[0] is not None:
                            add_dep_helper(mm.ins, last_proj[0].ins, False,
                                           "software-pipeline e-mm behind proj")
                    g, s = (j_f % 4) * 32, j_f // 4
                    nc.scalar.copy(
                        e_stage[g:g + 1, s * 512:(s + 1) * 512], e_ps_f[:])
                    if j_f == 15:
                        softmax_half(0)

                kr_tiles = {}

                def issue_kr(kk):
                    kr_t = kr_pool.tile([128, 4 * H], BF16, tag="kr",
                                        name=f"kr_t{kk}")
                    nc.sync.dma_start(kr_t[:], krE[kk * 128:(kk + 1) * 128, :])
                    kr_tiles[kk] = kr_t

                bht_t = None
                for j in range(NJ):
                    if j == 28:
                        for kk in range(4, 9):
                            issue_kr(kk)
                    if 2 <= j < 32:
                        issue_bhn(j - 2)
                    if j == 31:
                        issue_bhn(30)
                        issue_bhn(31)
                    if j % 4 == 0:
                        bht_t = [bht_pool.tile([128, 4 * 512], BF16,
                                               tag=f"bht{kc}",
                                               name=f"bht_t{j}_{kc}")
                                 for kc in range(4)]
                        for kc in range(4):
                            if j == 0:
                                # split the first group, spread across both
                                # DMA queues, so early slices land ASAP
                                eng0 = nc.sync if kc < 2 else nc.gpsimd
                                eng0.dma_start(
                                    bht_t[kc][:, 0:512],
                                    bhT[kc * 128:(kc + 1) * 128, 0:512],
                                )
                                eng0.dma_start(
                                    bht_t[kc][:, 512:2048],
                                    bhT[kc * 128:(kc + 1) * 128, 512:2048],
                                )
                            elif j == 4:
                                eng4 = nc.sync if kc < 2 else nc.gpsimd
                                for j2 in range(2):
                                    eng4.dma_start(
                                        bht_t[kc][:, j2 * 1024:(j2 + 1) * 1024],
                                        bhT[kc * 128:(kc + 1) * 128,
                                            (4 + j2 * 2) * 512:(6 + j2 * 2) * 512],
                                    )
                            else:
                                nc.sync.dma_start(
                                    bht_t[kc][:],
                                    bhT[kc * 128:(kc + 1) * 128,
                                        j * 512:(j + 4) * 512],
                                )
                    jj = j % 4
                    e_ps = psE.tile([1, 512], F32)
                    for hc in range(4):
                        proj_ps = psB.tile([128, 512], F32)
                        for kc in range(4):
                            last_proj[0] = nc.tensor.matmul(
                                proj_ps[:],
                                wi_sb[:, kc * H + hc * 128:kc * H + (hc + 1) * 128],
                                bht_t[kc][:, jj * 512:(jj + 1) * 512],
                                start=(kc == 0), stop=(kc == 3),
                            )
                        # hp bias-add on DVE (PSUM -> SBUF f32 staging), then a
                        # single whole-tile tanh on ACT
                        parg_t = parg_pool.tile([128, 512], F32)
                        for half in range(2):
                            b_idx = 2 * j + half
                            nc.vector.tensor_scalar_add(
                                parg_t[:, half * 256:(half + 1) * 256],
                                proj_ps[:, half * 256:(half + 1) * 256],
                                hpT_sb[:, hc * P + b_idx:hc * P + b_idx + 1],
                            )
                        tanh_t = tanh_pool.tile([128, 512], BF16)
                        nc.scalar.activation(tanh_t[:], parg_t[:], AF.Tanh)
                        pending.append((e_ps, hc, tanh_t, j))
                        if len(pending) >= 8:
                            flush_e()
                while pending:
                    flush_e()
                softmax_half(1)
                for kk in range(4):
                    issue_kr(kk)
                # bd half h only has nonzero columns for its own 32 b-rows, so
                # each ctx half-accumulation yields a disjoint row range: no
                # sum needed, and each half's transpose chain starts as soon
                # as its own matmuls finish (hiding under the other half / z)
                ctp = wpool.tile([P, H], BF16)
                for hi2 in range(2):
                    # ctx half 0's operands are ready the moment B drains
                    ctx_half(hi2)
                    rs2 = slice(32 * hi2, 32 * (hi2 + 1))
                    nc.vector.tensor_copy(ctx_bf[rs2, :], ctx_haps[hi2][rs2, :])
                    nc.vector.transpose(ctp[rs2, :], ctx_bf[rs2, :])
                    for v in range(16):
                        kk2 = v // 4
                        pbase = (v % 4) * 32
                        nc.vector.tensor_copy(
                            xs_ctx[pbase:pbase + 32,
                                   kk2 * P + 32 * hi2:kk2 * P + 32 * hi2 + 32],
                            ctp[rs2, 32 * v:32 * (v + 1)],
                        )

            # ---- phase E: LSTM cell ----
            # xstack split: the context-independent rows live in their own
            # tensor so the first z-matmuls don't wait on the ctx transpose
            xs_rest = wpool.tile([128, 5 * P], BF16)
            nc.sync.dma_start(xs_rest[:], xrt[:])

            gi = wpool.tile([P, H], F32)
            gf = wpool.tile([P, H], F32)
            gg = wpool.tile([P, H], F32)
            go = wpool.tile([P, H], F32)
            with tc.tile_pool(name="psZ", bufs=1, space="PSUM") as psZ:
                z_ps = psZ.tile([P, 4 * H], F32)
                # context-independent rows (onehots/prev_h/bias) first, so the
                # PE works while the context transpose chain completes
                for kk in list(range(4, 9)) + list(range(4)):
                    kr_t = kr_tiles[kk]
                    xsl = (xs_ctx[:, kk * P:(kk + 1) * P] if kk < 4 else
                           xs_rest[:, (kk - 4) * P:(kk - 3) * P])
                    for nn in range(4):
                        nc.tensor.matmul(
                            z_ps[:, nn * 512:(nn + 1) * 512],
                            xsl,
                            kr_t[:, nn * 512:(nn + 1) * 512],
                            start=(kk == 4), stop=(kk == 3),
                        )
                nc.scalar.activation(gi[:], z_ps[:, 0:512], AF.Sigmoid)
                nc.scalar.activation(gf[:], z_ps[:, 512:1024], AF.Sigmoid)
                nc.scalar.activation(gg[:], z_ps[:, 1024:1536], AF.Tanh)
                nc.scalar.activation(go[:], z_ps[:, 1536:2048], AF.Sigmoid)

            pc_sb = wpool.tile([P, H], F32)
            nc.sync.dma_start(pc_sb[:], pc[:])
            t1 = wpool.tile([P, H], F32)
            t2 = wpool.tile([P, H], F32)
            c_sb = wpool.tile([P, H], F32)
            nc.vector.tensor_mul(t1[:], gf[:], pc_sb[:])
            nc.vector.tensor_mul(t2[:], gi[:], gg[:])
            nc.vector.tensor_add(c_sb[:], t1[:], t2[:])
            nc.sync.dma_start(c_out[:], c_sb[:])
            tc_sb = wpool.tile([P, H], F32)
            nc.scalar.activation(tc_sb[:], c_sb[:], AF.Tanh)
            h_sb = wpool.tile([P, H], F32)
            nc.vector.tensor_mul(h_sb[:], go[:], tc_sb[:])
            nc.sync.dma_start(h_out[:], h_sb[:])

    nc.compile()
    return nc


def _get_nc():
    global _CACHED
    if _CACHED is None:
        _CACHED = _build()
    return _CACHED


def _prep_in_maps(inputs):
    batch_H = np.asarray(inputs["batch_H"], np.float32)
    prev_h = np.asarray(inputs["prev_h"], np.float32)
    prev_c = np.asarray(inputs["prev_c"], np.float32)
    onehots = np.asarray(inputs["char_onehots"], np.float32)
    Wi = np.asarray(inputs["Wi"], np.float32)
    Wh = np.asarray(inputs["Wh"], np.float32)
    bh = np.asarray(inputs["bh"], np.float32)
    Ws = np.asarray(inputs["Ws"], np.float32)
    K = np.asarray(inputs["K"], np.float32)
    R = np.asarray(inputs["R"], np.float32)
    b = np.asarray(inputs["b"], np.float32)

    def pack(a, k):  # [(k p), n] -> [p, (k n)]
        n = a.shape[1]
        return np.ascontiguousarray(
            a.reshape(k, 128, n).transpose(1, 0, 2).reshape(128, k * n))

    wi_b = pack(Wi, 4).astype(bf16)
    ws_b = np.ascontiguousarray(Ws.reshape(4, 128).T).astype(bf16)
    whE = np.zeros((640, H), np.float32)
    whE[:H] = Wh
    whE[H] = bh
    whE_b = pack(whE, 5).astype(bf16)
    krE = np.zeros((KXP, 4 * H), np.float32)
    krE[:C + NCLS] = K
    krE[C + NCLS:C + NCLS + H] = R
    krE[C + NCLS + H] = b
    krE_b = krE.astype(bf16)

    in_maps = []
    for i in range(NCORES):
        sl = slice(i * P, (i + 1) * P)
        bh_s = batch_H[sl].reshape(BT, C)
        ph_s = prev_h[sl]                       # [64, 512]
        phTe = np.zeros((640, P), np.float32)
        phTe[:H] = ph_s.T
        phTe[H] = 1.0
        xrt = np.zeros((640, P), np.float32)
        xrt[:NCLS] = onehots[sl].T
        xrt[NCLS:NCLS + H] = ph_s.T
        xrt[NCLS + H] = 1.0
        bhn_p = np.ascontiguousarray(
            bh_s.reshape(32, 4, 128, C).transpose(0, 2, 1, 3).reshape(BT // 4, 4 * C))
        in_maps.append({
            "bhT": np.ascontiguousarray(bh_s.T).astype(bf16),
            "bhN": bhn_p.astype(bf16),
            "wi": wi_b,
            "whE": whE_b,
            "phTe": pack(phTe, 5).astype(bf16),
            "ws": ws_b,
            "xrt": pack(xrt, 5).astype(bf16),
            "krE": krE_b,
            "pc": prev_c[sl],
        })
    return in_maps


def run(inputs, **run_kwargs):
    nc = _get_nc()
    in_maps = _prep_in_maps(inputs)
    res = run_bass_kernel_spmd(nc, in_maps, core_ids=list(range(NCORES)),
                               **run_kwargs)
    h = np.concatenate([res.results[i]["h_out"] for i in range(NCORES)], 0)
    c = np.concatenate([res.results[i]["c_out"] for i in range(NCORES)], 0)
    alpha = np.concatenate([res.results[i]["alpha_out"] for i in range(NCORES)], 0)
    return (h, c, alpha.reshape(B, T, 1)), res


def kernel(**inputs):
    (h, c, alpha), _ = run(inputs)
    return (h, c, alpha)


# revision 40
# speedup vs baseline: 1.0606x; 1.0136x over previous
"""Trainium2 Bass kernel for nn_AttentionCell (Bahdanau attention + LSTM cell step).

Data-parallel over batch across 8 NeuronCores: each core gets B/8 = 64 batch
rows (all weights replicated) and computes its slice of (h_new, c_new, alpha)
independently — no collectives.

Per-core pipeline (shard b = 64, BT = b*T = 16384, all matmuls bf16 / f32 acc):
  phase A: hp^T = Wh^T prev_h^T + bh                        (PE, tiny)
  phase B: proj^T tiles [128h, 512bt] = Wi^T @ batch_H^T    (PE)
           + hp bias on DVE, whole-tile tanh on ACT -> bf16
           e = Ws^T tanh-tiles (M=1 matmuls, software-pipelined ~2 j behind
           and emitted per-j in batches of 4 to amortize the PE's 128<->32
           tile-size reconfig)
  phase C: softmax over t, split into two b-halves emitted INSIDE phase B so
           the serial chain hides under the tail of B; the block-diagonal
           alpha operand is built with DVE 32x32 transposes + strided copies
  phase D: context[64, 512] = alpha_blockdiag^T @ batch_H (one 128-matmul
           PSUM accumulation); the first 64 matmuls are interleaved into
           phase B's tail so the PE never idles or goes HAM-cold
  phase E: z = [context|onehots|prev_h|1] @ [K;R;b] -> LSTM gates -> h, c
           (the context-independent contraction rows run first while the
           context transpose is still in flight)

batch_H is passed from the host in BOTH layouts (transposed for phase B,
natural for phase D), pre-cast to bf16 and pre-packed so every load is a
single contiguous 2D DMA.
"""

import numpy as np
import ml_dtypes

import concourse.bass as bass
import concourse.mybir as mybir
import concourse.tile as tile
from concourse import bacc
from concourse.bass_utils import run_bass_kernel_spmd
from concourse.tile_rust import add_dep_helper

bf16 = ml_dtypes.bfloat16
F32 = mybir.dt.float32
BF16 = mybir.dt.bfloat16
AF = mybir.ActivationFunctionType
ALU = mybir.AluOpType
AX = mybir.AxisListType

NCORES = 8
B, T, C, H, NCLS = 512, 256, 512, 512, 96
P = B // NCORES          # 64 batch rows per core
BT = P * T               # 16384 flattened (b, t) rows per core
KXP = 1152               # LSTM contraction rows (1121) padded to 9*128
NJ = BT // 512           # 32 bt-chunks of 512
NKT = BT // 128          # 128 bt-chunks of 128

_CACHED = None


def _build():
    nc = bacc.Bacc("TRN2", target_bir_lowering=False, debug=False,
                   num_devices=NCORES)

    bhT = nc.dram_tensor("bhT", [C, BT], BF16, kind="ExternalInput")
    bhN = nc.dram_tensor("bhN", [BT // 4, 4 * C], BF16, kind="ExternalInput")
    wi = nc.dram_tensor("wi", [128, 4 * H], BF16, kind="ExternalInput")
    whE = nc.dram_tensor("whE", [128, 5 * H], BF16, kind="ExternalInput")
    phTe = nc.dram_tensor("phTe", [128, 5 * P], BF16, kind="ExternalInput")
    ws = nc.dram_tensor("ws", [128, 4 * 128], BF16, kind="ExternalInput")
    xrt = nc.dram_tensor("xrt", [128, 5 * P], BF16, kind="ExternalInput")
    krE = nc.dram_tensor("krE", [KXP, 4 * H], BF16, kind="ExternalInput")
    pc = nc.dram_tensor("pc", [P, H], F32, kind="ExternalInput")

    e_tmp = nc.dram_tensor("e_tmp", [BT], F32)  # internal staging for reshape
    h_out = nc.dram_tensor("h_out", [P, H], F32, kind="ExternalOutput")
    c_out = nc.dram_tensor("c_out", [P, H], F32, kind="ExternalOutput")
    alpha_out = nc.dram_tensor("alpha_out", [P, T], F32, kind="ExternalOutput")

    with tile.TileContext(nc) as tc:
        with (
            tc.tile_pool(name="const", bufs=1) as cpool,
            tc.tile_pool(name="work", bufs=1) as wpool,
            tc.tile_pool(name="bht", bufs=3) as bht_pool,
            tc.tile_pool(name="tanh", bufs=8) as tanh_pool,
            tc.tile_pool(name="parg", bufs=3) as parg_pool,
            tc.tile_pool(name="bhn", bufs=13) as bhn_pool,
            tc.tile_pool(name="kr", bufs=5) as kr_pool,
        ):
            # ---- constants / weights into SBUF ----
            whE_sb = cpool.tile([128, 5 * H], BF16)
            nc.gpsimd.dma_start(whE_sb[:], whE[:])
            phTe_sb = cpool.tile([128, 5 * P], BF16)
            nc.gpsimd.dma_start(phTe_sb[:], phTe[:])
            wi_sb = cpool.tile([128, 4 * H], BF16)           # [p, (kc h)]
            nc.gpsimd.dma_start(wi_sb[:], wi[:])
            # ws replicated into 128 columns: the e-matmul then uses the
            # same (128,128) tile_size as proj -> no PE reconfig penalty
            ws_sb = cpool.tile([128, 4 * 128], BF16)
            nc.gpsimd.dma_start(ws_sb[:], ws[:])

            # block-diagonal alpha operand, one tensor per b-half so ctx
            # half 0 doesn't falsely depend on half 1's writers
            bd0 = wpool.tile([128, 64 * P], BF16)
            nc.vector.memset(bd0[:], 0.0)
            bd1 = wpool.tile([128, 64 * P], BF16)
            nc.vector.memset(bd1[:], 0.0)
            bd_half = [bd0, bd1]

            # phase-D rhs tiles arrive on the SWDGE queue (4 kt rows per DMA);
            # the first few are issued up front, the rest are spread through
            # the j-loop so they don't starve the bhT loads early on
            bhn_t = []

            def issue_bhn(g):
                bt_g = bhn_pool.tile([128, 4 * C], BF16, tag="bhn",
                                     name=f"bhn_t{g}")
                nc.gpsimd.dma_start(bt_g[:], bhN[g * 128:(g + 1) * 128, :])
                bhn_t.append(bt_g)


            # ---- phase A: hp^T [128h x 4hc, 64b] (bh folded via ones row) ----
            hpT_sb = wpool.tile([128, 4 * P], F32)
            with tc.tile_pool(name="psA", bufs=2, space="PSUM") as psA:
                for hc in range(4):
                    hp_ps = psA.tile([128, P], F32)
                    for kk in range(5):
                        nc.tensor.matmul(
                            hp_ps[:],
                            whE_sb[:, kk * H + hc * 128:kk * H + (hc + 1) * 128],
                            phTe_sb[:, kk * P:(kk + 1) * P],
                            start=(kk == 0), stop=(kk == 4),
                        )
                    nc.vector.tensor_copy(hpT_sb[:, hc * P:(hc + 1) * P], hp_ps[:])

            # ---- phases B + C + D (fused) ----
            e_stage = wpool.tile([128, (NJ // 4) * 512], F32)
            e_bt = wpool.tile([P, T], F32)
            mx_neg = wpool.tile([P, 1], F32)
            ex = wpool.tile([P, T], F32)
            ssum = wpool.tile([P, 1], F32)
            rsum = wpool.tile([P, 1], F32)
            alpha_sb = wpool.tile([P, T], F32)
            at0 = wpool.tile([32, T], F32)
            at1 = wpool.tile([32, T], F32)
            atp = [at0, at1]
            ctx_bf = wpool.tile([P, H], BF16)
            xs_ctx = wpool.tile([128, 4 * P], BF16)

            def softmax_half(hi):
                # e rows for b in [32*hi, 32*hi+32) live in slots s=4*hi..4*hi+3
                lo = hi * (BT // 2)
                ew = nc.sync.dma_start(
                    e_tmp[lo:lo + BT // 2].rearrange("(s g n) -> g s n", s=4, g=4),
                    e_stage[0:128:32, hi * 2048:(hi + 1) * 2048].rearrange(
                        "g (s n) -> g s n", s=4),
                )
                sl = slice(32 * hi, 32 * (hi + 1))
                er = nc.sync.dma_start(
                    e_bt[sl, :],
                    e_tmp[lo:lo + BT // 2].rearrange("(b t) -> b t", b=32))
                add_dep_helper(er.ins, ew.ins, True, f"e_tmp half{hi} order")
                nc.vector.tensor_reduce(mx_neg[sl, :], e_bt[sl, :], AX.X,
                                        ALU.max, negate=True)
                nc.scalar.activation(ex[sl, :], e_bt[sl, :], AF.Exp,
                                     bias=mx_neg[sl, :])
                nc.vector.tensor_reduce(ssum[sl, :], ex[sl, :], AX.X, ALU.add)
                nc.vector.reciprocal(rsum[sl, :], ssum[sl, :])
                nc.vector.tensor_scalar_mul(alpha_sb[sl, :], ex[sl, :], rsum[sl, :])
                nc.sync.dma_start(alpha_out[sl, :], alpha_sb[sl, :])
                # block-diagonal build: DVE 32x32 block transpose, then one
                # strided copy per t-block.  bd[p, (2b+i)*64 + b] = alpha[b, t]
                # with t = i*128 + p  ->  free offset 129*b + 64*i, stride 129
                at = atp[hi]
                nc.vector.transpose(at[:], alpha_sb[sl, :])
                for jb in range(8):
                    ihalf = jb // 4
                    pbase = (jb % 4) * 32
                    off = 64 * ihalf + 32 * hi
                    nc.vector.tensor_copy(
                        bd_half[hi][pbase:pbase + 32,
                                    off:off + 129 * 31 + 1:129],
                        at[:, 32 * jb:32 * (jb + 1)],
                    )

            with tc.tile_pool(name="psD", bufs=2, space="PSUM") as psD:
              with (
                tc.tile_pool(name="psB", bufs=4, space="PSUM") as psB,
                tc.tile_pool(name="psE", bufs=2, space="PSUM") as psE,
              ):
                ctx_haps = [psD.tile([P, H], F32, tag="ctx", name="ctx_ps0"),
                            psD.tile([P, H], F32, tag="ctx", name="ctx_ps1")]

                def ctx_half(hi):
                    # each half is a self-contained accumulation group in its
                    # own PSUM bank (a group split across other matmul groups
                    # wedges the exec unit); the two banks are summed on DVE
                    for kt in range(64 * hi, 64 * (hi + 1)):
                        g, i = kt // 4, kt % 4
                        nc.tensor.matmul(
                            ctx_haps[hi][:],
                            bd_half[hi][:, (kt % 64) * P:(kt % 64 + 1) * P],
                            bhn_t[g][:, i * C:(i + 1) * C],
                            start=(kt % 64 == 0), stop=(kt % 64 == 63),
                        )

                # e-matmuls are software-pipelined ~2 j behind their tanh
                # producers and emitted per-j in batches of 4
                pending = []
                last_proj = [None]

                def flush_e():
                    for _ in range(4):
                        e_ps_f, hc_f, tanh_f, j_f = pending.pop(0)
                        mm = nc.tensor.matmul(
                            e_ps_f[:],
                            ws_sb[:, hc_f * 128:(hc_f + 1) * 128],
                            tanh_f[:],
                            start=(hc_f == 0), stop=(hc_f == 3),
                        )
                        if last_proj[0] is not None:
                            add_dep_helper(mm.ins, last_proj[0].ins, False,
                                           "software-pipeline e-mm behind proj")
                    g, s = (j_f % 4) * 32, j_f // 4
                    nc.scalar.copy(
                        e_stage[g:g + 1, s * 512:(s + 1) * 512], e_ps_f[0:1, :])
                    if j_f == 15:
                        softmax_half(0)

                kr_tiles = {}

                def issue_kr(kk):
                    kr_t = kr_pool.tile([128, 4 * H], BF16, tag="kr",
                                        name=f"kr_t{kk}")
                    nc.sync.dma_start(kr_t[:], krE[kk * 128:(kk + 1) * 128, :])
                    kr_tiles[kk] = kr_t

                bht_t = None
                for j in range(NJ):
                    if j == 28:
                        for kk in range(4, 9):
                            issue_kr(kk)
                    if 2 <= j < 32:
                        issue_bhn(j - 2)
                    if j == 31:
                        issue_bhn(30)
                        issue_bhn(31)
                    if j % 4 == 0:
                        bht_t = [bht_pool.tile([128, 4 * 512], BF16,
                                               tag=f"bht{kc}",
                                               name=f"bht_t{j}_{kc}")
                                 for kc in range(4)]
                        for kc in range(4):
                            if j == 0:
                                # split the first group, spread across both
                                # DMA queues, so early slices land ASAP
                                eng0 = nc.sync if kc < 2 else nc.gpsimd
                                eng0.dma_start(
                                    bht_t[kc][:, 0:512],
                                    bhT[kc * 128:(kc + 1) * 128, 0:512],
                                )
                                eng0.dma_start(
                                    bht_t[kc][:, 512:2048],
                                    bhT[kc * 128:(kc + 1) * 128, 512:2048],
                                )
                            elif j == 4:
                                eng4 = nc.sync if kc < 2 else nc.gpsimd
                                for j2 in range(2):
                                    eng4.dma_start(
                                        bht_t[kc][:, j2 * 1024:(j2 + 1) * 1024],
                                        bhT[kc * 128:(kc + 1) * 128,
                                            (4 + j2 * 2) * 512:(6 + j2 * 2) * 512],
                                    )
                            else:
                                nc.sync.dma_start(
                                    bht_t[kc][:],
                                    bhT[kc * 128:(kc + 1) * 128,
                                        j * 512:(j + 4) * 512],
                                )
                    jj = j % 4
                    e_ps = psE.tile([128, 512], F32)
                    for hc in range(4):
                        proj_ps = psB.tile([128, 512], F32)
                        for kc in range(4):
                            last_proj[0] = nc.tensor.matmul(
                                proj_ps[:],
                                wi_sb[:, kc * H + hc * 128:kc * H + (hc + 1) * 128],
                                bht_t[kc][:, jj * 512:(jj + 1) * 512],
                                start=(kc == 0), stop=(kc == 3),
                            )
                        # hp bias-add on DVE (PSUM -> SBUF f32 staging), then a
                        # single whole-tile tanh on ACT
                        parg_t = parg_pool.tile([128, 512], F32)
                        for half in range(2):
                            b_idx = 2 * j + half
                            nc.vector.tensor_scalar_add(
                                parg_t[:, half * 256:(half + 1) * 256],
                                proj_ps[:, half * 256:(half + 1) * 256],
                                hpT_sb[:, hc * P + b_idx:hc * P + b_idx + 1],
                            )
                        tanh_t = tanh_pool.tile([128, 512], BF16)
                        nc.scalar.activation(tanh_t[:], parg_t[:], AF.Tanh)
                        pending.append((e_ps, hc, tanh_t, j))
                        if len(pending) >= 8:
                            flush_e()
                while pending:
                    flush_e()
              # psB/psE released here so psZ's banks free at B-end, not ctx-end
              if True:
                softmax_half(1)
                for kk in range(4):
                    issue_kr(kk)
                # bd half h only has nonzero columns for its own 32 b-rows, so
                # each ctx half-accumulation yields a disjoint row range: no
                # sum needed, and each half's transpose chain starts as soon
                # as its own matmuls finish (hiding under the other half / z)
                ctp = wpool.tile([P, H], BF16)
                for hi2 in range(2):
                    # ctx half 0's operands are ready the moment B drains
                    ctx_half(hi2)
                    rs2 = slice(32 * hi2, 32 * (hi2 + 1))
                    nc.vector.tensor_copy(ctx_bf[rs2, :], ctx_haps[hi2][rs2, :])
                    nc.vector.transpose(ctp[rs2, :], ctx_bf[rs2, :])
                    for v in range(16):
                        kk2 = v // 4
                        pbase = (v % 4) * 32
                        nc.vector.tensor_copy(
                            xs_ctx[pbase:pbase + 32,
                                   kk2 * P + 32 * hi2:kk2 * P + 32 * hi2 + 32],
                            ctp[rs2, 32 * v:32 * (v + 1)],
                        )

            # ---- phase E: LSTM cell ----
            # xstack split: the context-independent rows live in their own
            # tensor so the first z-matmuls don't wait on the ctx transpose
            xs_rest = wpool.tile([128, 5 * P], BF16)
            nc.sync.dma_start(xs_rest[:], xrt[:])

            gi = wpool.tile([P, H], F32)
            gf = wpool.tile([P, H], F32)
            gg = wpool.tile([P, H], F32)
            go = wpool.tile([P, H], F32)
            with tc.tile_pool(name="psZ", bufs=1, space="PSUM") as psZ:
                z_ps = psZ.tile([P, 4 * H], F32)
                # context-independent rows (onehots/prev_h/bias) first, so the
                # PE works while the context transpose chain completes
                for kk in list(range(4, 9)) + list(range(4)):
                    kr_t = kr_tiles[kk]
                    xsl = (xs_ctx[:, kk * P:(kk + 1) * P] if kk < 4 else
                           xs_rest[:, (kk - 4) * P:(kk - 3) * P])
                    for nn in range(4):
                        nc.tensor.matmul(
                            z_ps[:, nn * 512:(nn + 1) * 512],
                            xsl,
                            kr_t[:, nn * 512:(nn + 1) * 512],
                            start=(kk == 4), stop=(kk == 3),
                        )
                nc.scalar.activation(gi[:], z_ps[:, 0:512], AF.Sigmoid)
                nc.scalar.activation(gf[:], z_ps[:, 512:1024], AF.Sigmoid)
                nc.scalar.activation(gg[:], z_ps[:, 1024:1536], AF.Tanh)
                nc.scalar.activation(go[:], z_ps[:, 1536:2048], AF.Sigmoid)

            pc_sb = wpool.tile([P, H], F32)
            nc.sync.dma_start(pc_sb[:], pc[:])
            t1 = wpool.tile([P, H], F32)
            t2 = wpool.tile([P, H], F32)
            c_sb = wpool.tile([P, H], F32)
            nc.vector.tensor_mul(t1[:], gf[:], pc_sb[:])
            nc.vector.tensor_mul(t2[:], gi[:], gg[:])
            nc.vector.tensor_add(c_sb[:], t1[:], t2[:])
            nc.sync.dma_start(c_out[:], c_sb[:])
            tc_sb = wpool.tile([P, H], F32)
            nc.scalar.activation(tc_sb[:], c_sb[:], AF.Tanh)
            h_sb = wpool.tile([P, H], F32)
            nc.vector.tensor_mul(h_sb[:], go[:], tc_sb[:])
            nc.sync.dma_start(h_out[:], h_sb[:])

    nc.compile()
    return nc


def _get_nc():
    global _CACHED
    if _CACHED is None:
        _CACHED = _build()
    return _CACHED


def _prep_in_maps(inputs):
    batch_H = np.asarray(inputs["batch_H"], np.float32)
    prev_h = np.asarray(inputs["prev_h"], np.float32)
    prev_c = np.asarray(inputs["prev_c"], np.float32)
    onehots = np.asarray(inputs["char_onehots"], np.float32)
    Wi = np.asarray(inputs["Wi"], np.float32)
    Wh = np.asarray(inputs["Wh"], np.float32)
    bh = np.asarray(inputs["bh"], np.float32)
    Ws = np.asarray(inputs["Ws"], np.float32)
    K = np.asarray(inputs["K"], np.float32)
    R = np.asarray(inputs["R"], np.float32)
    b = np.asarray(inputs["b"], np.float32)

    def pack(a, k):  # [(k p), n] -> [p, (k n)]
        n = a.shape[1]
        return np.ascontiguousarray(
            a.reshape(k, 128, n).transpose(1, 0, 2).reshape(128, k * n))

    wi_b = pack(Wi, 4).astype(bf16)
    ws_cols = np.ascontiguousarray(Ws.reshape(4, 128).T)      # [p, kc]
    ws_b = np.repeat(ws_cols[:, :, None], 128, axis=2).reshape(128, 512).astype(bf16)
    whE = np.zeros((640, H), np.float32)
    whE[:H] = Wh
    whE[H] = bh
    whE_b = pack(whE, 5).astype(bf16)
    krE = np.zeros((KXP, 4 * H), np.float32)
    krE[:C + NCLS] = K
    krE[C + NCLS:C + NCLS + H] = R
    krE[C + NCLS + H] = b
    krE_b = krE.astype(bf16)

    in_maps = []
    for i in range(NCORES):
        sl = slice(i * P, (i + 1) * P)
        bh_s = batch_H[sl].reshape(BT, C)
        ph_s = prev_h[sl]                       # [64, 512]
        phTe = np.zeros((640, P), np.float32)
        phTe[:H] = ph_s.T
        phTe[H] = 1.0
        xrt = np.zeros((640, P), np.float32)
        xrt[:NCLS] = onehots[sl].T
        xrt[NCLS:NCLS + H] = ph_s.T
        xrt[NCLS + H] = 1.0
        bhn_p = np.ascontiguousarray(
            bh_s.reshape(32, 4, 128, C).transpose(0, 2, 1, 3).reshape(BT // 4, 4 * C))
        in_maps.append({
            "bhT": np.ascontiguousarray(bh_s.T).astype(bf16),
            "bhN": bhn_p.astype(bf16),
            "wi": wi_b,
            "whE": whE_b,
            "phTe": pack(phTe, 5).astype(bf16),
            "ws": ws_b,
            "xrt": pack(xrt, 5).astype(bf16),
            "krE": krE_b,
            "pc": prev_c[sl],
        })
    return in_maps


def run(inputs, **run_kwargs):
    nc = _get_nc()
    in_maps = _prep_in_maps(inputs)
    res = run_bass_kernel_spmd(nc, in_maps, core_ids=list(range(NCORES)),
                               **run_kwargs)
    h = np.concatenate([res.results[i]["h_out"] for i in range(NCORES)], 0)
    c = np.concatenate([res.results[i]["c_out"] for i in range(NCORES)], 0)
    alpha = np.concatenate([res.results[i]["alpha_out"] for i in range(NCORES)], 0)
    return (h, c, alpha.reshape(B, T, 1)), res


def kernel(**inputs):
    (h, c, alpha), _ = run(inputs)
    return (h, c, alpha)


# revision 41
# speedup vs baseline: 1.0646x; 1.0037x over previous
"""Trainium2 Bass kernel for nn_AttentionCell (Bahdanau attention + LSTM cell step).

Data-parallel over batch across 8 NeuronCores: each core gets B/8 = 64 batch
rows (all weights replicated) and computes its slice of (h_new, c_new, alpha)
independently — no collectives.

Per-core pipeline (shard b = 64, BT = b*T = 16384, all matmuls bf16 / f32 acc):
  phase A: hp^T = Wh^T prev_h^T + bh                        (PE, tiny)
  phase B: proj^T tiles [128h, 512bt] = Wi^T @ batch_H^T    (PE)
           + hp bias on DVE, whole-tile tanh on ACT -> bf16
           e = Ws^T tanh-tiles (M=1 matmuls, software-pipelined ~2 j behind
           and emitted per-j in batches of 4 to amortize the PE's 128<->32
           tile-size reconfig)
  phase C: softmax over t, split into two b-halves emitted INSIDE phase B so
           the serial chain hides under the tail of B; the block-diagonal
           alpha operand is built with DVE 32x32 transposes + strided copies
  phase D: context[64, 512] = alpha_blockdiag^T @ batch_H (one 128-matmul
           PSUM accumulation); the first 64 matmuls are interleaved into
           phase B's tail so the PE never idles or goes HAM-cold
  phase E: z = [context|onehots|prev_h|1] @ [K;R;b] -> LSTM gates -> h, c
           (the context-independent contraction rows run first while the
           context transpose is still in flight)

batch_H is passed from the host in BOTH layouts (transposed for phase B,
natural for phase D), pre-cast to bf16 and pre-packed so every load is a
single contiguous 2D DMA.
"""

import numpy as np
import ml_dtypes

import concourse.bass as bass
import concourse.mybir as mybir
import concourse.tile as tile
from concourse import bacc
from concourse.bass_utils import run_bass_kernel_spmd
from concourse.tile_rust import add_dep_helper

bf16 = ml_dtypes.bfloat16
F32 = mybir.dt.float32
BF16 = mybir.dt.bfloat16
AF = mybir.ActivationFunctionType
ALU = mybir.AluOpType
AX = mybir.AxisListType

NCORES = 8
B, T, C, H, NCLS = 512, 256, 512, 512, 96
P = B // NCORES          # 64 batch rows per core
BT = P * T               # 16384 flattened (b, t) rows per core
KXP = 1152               # LSTM contraction rows (1121) padded to 9*128
NJ = BT // 512           # 32 bt-chunks of 512
NKT = BT // 128          # 128 bt-chunks of 128

_CACHED = None


def _build():
    nc = bacc.Bacc("TRN2", target_bir_lowering=False, debug=False,
                   num_devices=NCORES)

    bhT = nc.dram_tensor("bhT", [C, BT], BF16, kind="ExternalInput")
    bhN = nc.dram_tensor("bhN", [BT // 4, 4 * C], BF16, kind="ExternalInput")
    wi = nc.dram_tensor("wi", [128, 4 * H], BF16, kind="ExternalInput")
    whE = nc.dram_tensor("whE", [128, 5 * H], BF16, kind="ExternalInput")
    phTe = nc.dram_tensor("phTe", [128, 5 * P], BF16, kind="ExternalInput")
    ws = nc.dram_tensor("ws", [128, 4 * 128], BF16, kind="ExternalInput")
    xrt = nc.dram_tensor("xrt", [128, 5 * P], BF16, kind="ExternalInput")
    krE = nc.dram_tensor("krE", [KXP, 4 * H], BF16, kind="ExternalInput")
    pc = nc.dram_tensor("pc", [P, H], F32, kind="ExternalInput")

    e_tmp = nc.dram_tensor("e_tmp", [BT], F32)  # internal staging for reshape
    h_out = nc.dram_tensor("h_out", [P, H], F32, kind="ExternalOutput")
    c_out = nc.dram_tensor("c_out", [P, H], F32, kind="ExternalOutput")
    alpha_out = nc.dram_tensor("alpha_out", [P, T], F32, kind="ExternalOutput")

    with tile.TileContext(nc) as tc:
        with (
            tc.tile_pool(name="const", bufs=1) as cpool,
            tc.tile_pool(name="work", bufs=1) as wpool,
            tc.tile_pool(name="bht", bufs=3) as bht_pool,
            tc.tile_pool(name="tanh", bufs=8) as tanh_pool,
            tc.tile_pool(name="parg", bufs=3) as parg_pool,
            tc.tile_pool(name="bhn", bufs=13) as bhn_pool,
            tc.tile_pool(name="kr", bufs=5) as kr_pool,
        ):
            # ---- constants / weights into SBUF ----
            whE_sb = cpool.tile([128, 5 * H], BF16)
            nc.gpsimd.dma_start(whE_sb[:], whE[:])
            phTe_sb = cpool.tile([128, 5 * P], BF16)
            nc.gpsimd.dma_start(phTe_sb[:], phTe[:])
            wi_sb = cpool.tile([128, 4 * H], BF16)           # [p, (kc h)]
            nc.gpsimd.dma_start(wi_sb[:], wi[:])
            # ws replicated into 128 columns: the e-matmul then uses the
            # same (128,128) tile_size as proj -> no PE reconfig penalty
            ws_sb = cpool.tile([128, 4 * 128], BF16)
            nc.gpsimd.dma_start(ws_sb[:], ws[:])

            # block-diagonal alpha operand, one tensor per b-half so ctx
            # half 0 doesn't falsely depend on half 1's writers
            bd0 = wpool.tile([128, 64 * P], BF16)
            nc.vector.memset(bd0[:], 0.0)
            bd1 = wpool.tile([128, 64 * P], BF16)
            nc.vector.memset(bd1[:], 0.0)
            bd_half = [bd0, bd1]

            # phase-D rhs tiles arrive on the SWDGE queue (4 kt rows per DMA);
            # the first few are issued up front, the rest are spread through
            # the j-loop so they don't starve the bhT loads early on
            bhn_t = []

            def issue_bhn(g):
                bt_g = bhn_pool.tile([128, 4 * C], BF16, tag="bhn",
                                     name=f"bhn_t{g}")
                nc.gpsimd.dma_start(bt_g[:], bhN[g * 128:(g + 1) * 128, :])
                bhn_t.append(bt_g)


            # ---- phase A: hp^T [128h x 4hc, 64b] (bh folded via ones row) ----
            hpT_sb = wpool.tile([128, 4 * P], F32)
            with tc.tile_pool(name="psA", bufs=2, space="PSUM") as psA:
                for hc in range(4):
                    hp_ps = psA.tile([128, P], F32)
                    for kk in range(5):
                        nc.tensor.matmul(
                            hp_ps[:],
                            whE_sb[:, kk * H + hc * 128:kk * H + (hc + 1) * 128],
                            phTe_sb[:, kk * P:(kk + 1) * P],
                            start=(kk == 0), stop=(kk == 4),
                        )
                    nc.vector.tensor_copy(hpT_sb[:, hc * P:(hc + 1) * P], hp_ps[:])

            # ---- phases B + C + D (fused) ----
            e_stage = wpool.tile([128, (NJ // 4) * 512], F32)
            e_bt = wpool.tile([P, T], F32)
            mx_neg = wpool.tile([P, 1], F32)
            ex = wpool.tile([P, T], F32)
            ssum = wpool.tile([P, 1], F32)
            rsum = wpool.tile([P, 1], F32)
            alpha_sb = wpool.tile([P, T], F32)
            at0 = wpool.tile([32, T], F32)
            at1 = wpool.tile([32, T], F32)
            atp = [at0, at1]
            ctx_bf = wpool.tile([P, H], BF16)
            xs_ctx = wpool.tile([128, 4 * P], BF16)

            def softmax_half(hi):
                # e rows for b in [32*hi, 32*hi+32) live in slots s=4*hi..4*hi+3
                lo = hi * (BT // 2)
                ew = nc.sync.dma_start(
                    e_tmp[lo:lo + BT // 2].rearrange("(s g n) -> g s n", s=4, g=4),
                    e_stage[0:128:32, hi * 2048:(hi + 1) * 2048].rearrange(
                        "g (s n) -> g s n", s=4),
                )
                sl = slice(32 * hi, 32 * (hi + 1))
                er = nc.sync.dma_start(
                    e_bt[sl, :],
                    e_tmp[lo:lo + BT // 2].rearrange("(b t) -> b t", b=32))
                add_dep_helper(er.ins, ew.ins, True, f"e_tmp half{hi} order")
                nc.vector.tensor_reduce(mx_neg[sl, :], e_bt[sl, :], AX.X,
                                        ALU.max, negate=True)
                nc.scalar.activation(ex[sl, :], e_bt[sl, :], AF.Exp,
                                     bias=mx_neg[sl, :])
                nc.vector.tensor_reduce(ssum[sl, :], ex[sl, :], AX.X, ALU.add)
                nc.vector.reciprocal(rsum[sl, :], ssum[sl, :])
                nc.vector.tensor_scalar_mul(alpha_sb[sl, :], ex[sl, :], rsum[sl, :])
                nc.sync.dma_start(alpha_out[sl, :], alpha_sb[sl, :])
                # block-diagonal build: DVE 32x32 block transpose, then one
                # strided copy per t-block.  bd[p, (2b+i)*64 + b] = alpha[b, t]
                # with t = i*128 + p  ->  free offset 129*b + 64*i, stride 129
                at = atp[hi]
                nc.vector.transpose(at[:], alpha_sb[sl, :])
                for jb in range(8):
                    ihalf = jb // 4
                    pbase = (jb % 4) * 32
                    off = 64 * ihalf + 32 * hi
                    nc.vector.tensor_copy(
                        bd_half[hi][pbase:pbase + 32,
                                    off:off + 129 * 31 + 1:129],
                        at[:, 32 * jb:32 * (jb + 1)],
                    )

            with tc.tile_pool(name="psD", bufs=2, space="PSUM") as psD:
              with (
                tc.tile_pool(name="psB", bufs=4, space="PSUM") as psB,
                tc.tile_pool(name="psE", bufs=2, space="PSUM") as psE,
              ):
                ctx_haps = [psD.tile([P, H], F32, tag="ctx", name="ctx_ps0"),
                            psD.tile([P, H], F32, tag="ctx", name="ctx_ps1")]

                def ctx_half(hi):
                    # each half is a self-contained accumulation group in its
                    # own PSUM bank (a group split across other matmul groups
                    # wedges the exec unit); the two banks are summed on DVE
                    for kt in range(64 * hi, 64 * (hi + 1)):
                        g, i = kt // 4, kt % 4
                        nc.tensor.matmul(
                            ctx_haps[hi][:],
                            bd_half[hi][:, (kt % 64) * P:(kt % 64 + 1) * P],
                            bhn_t[g][:, i * C:(i + 1) * C],
                            start=(kt % 64 == 0), stop=(kt % 64 == 63),
                        )

                # e-matmuls are software-pipelined ~2 j behind their tanh
                # producers and emitted per-j in batches of 4
                pending = []
                last_proj = [None]

                def flush_e():
                    for _ in range(4):
                        e_ps_f, hc_f, tanh_f, j_f = pending.pop(0)
                        mm = nc.tensor.matmul(
                            e_ps_f[:],
                            ws_sb[:, hc_f * 128:(hc_f + 1) * 128],
                            tanh_f[:],
                            start=(hc_f == 0), stop=(hc_f == 3),
                        )
                        if last_proj[0] is not None:
                            add_dep_helper(mm.ins, last_proj[0].ins, False,
                                           "software-pipeline e-mm behind proj")
                    g, s = (j_f % 4) * 32, j_f // 4
                    nc.scalar.copy(
                        e_stage[g:g + 1, s * 512:(s + 1) * 512], e_ps_f[0:1, :])
                    if j_f == 15:
                        softmax_half(0)

                kr_tiles = {}

                def issue_kr(kk):
                    kr_t = kr_pool.tile([128, 4 * H], BF16, tag="kr",
                                        name=f"kr_t{kk}")
                    nc.sync.dma_start(kr_t[:], krE[kk * 128:(kk + 1) * 128, :])
                    kr_tiles[kk] = kr_t

                bht_t = None
                for j in range(NJ):
                    if j == 28:
                        for kk in range(4, 9):
                            issue_kr(kk)
                    if 2 <= j < 32:
                        issue_bhn(j - 2)
                    if j == 31:
                        issue_bhn(30)
                        issue_bhn(31)
                    if j % 4 == 0:
                        bht_t = [bht_pool.tile([128, 4 * 512], BF16,
                                               tag=f"bht{kc}",
                                               name=f"bht_t{j}_{kc}")
                                 for kc in range(4)]
                        for kc in range(4):
                            if j == 0:
                                # split the first group, spread across both
                                # DMA queues, so early slices land ASAP
                                eng0 = nc.sync if kc < 2 else nc.gpsimd
                                eng0.dma_start(
                                    bht_t[kc][:, 0:512],
                                    bhT[kc * 128:(kc + 1) * 128, 0:512],
                                )
                                eng0.dma_start(
                                    bht_t[kc][:, 512:2048],
                                    bhT[kc * 128:(kc + 1) * 128, 512:2048],
                                )
                            elif j == 4:
                                eng4 = nc.sync if kc < 2 else nc.gpsimd
                                for j2 in range(2):
                                    eng4.dma_start(
                                        bht_t[kc][:, j2 * 1024:(j2 + 1) * 1024],
                                        bhT[kc * 128:(kc + 1) * 128,
                                            (4 + j2 * 2) * 512:(6 + j2 * 2) * 512],
                                    )
                            else:
                                nc.sync.dma_start(
                                    bht_t[kc][:],
                                    bhT[kc * 128:(kc + 1) * 128,
                                        j * 512:(j + 4) * 512],
                                )
                    jj = j % 4
                    e_ps = psE.tile([128, 512], F32)
                    for hc in range(4):
                        proj_ps = psB.tile([128, 512], F32)
                        for kc in range(4):
                            last_proj[0] = nc.tensor.matmul(
                                proj_ps[:],
                                wi_sb[:, kc * H + hc * 128:kc * H + (hc + 1) * 128],
                                bht_t[kc][:, jj * 512:(jj + 1) * 512],
                                start=(kc == 0), stop=(kc == 3),
                            )
                        # hp bias-add on DVE (PSUM -> SBUF f32 staging), then a
                        # single whole-tile tanh on ACT
                        parg_t = parg_pool.tile([128, 512], F32)
                        for half in range(2):
                            b_idx = 2 * j + half
                            nc.vector.tensor_scalar_add(
                                parg_t[:, half * 256:(half + 1) * 256],
                                proj_ps[:, half * 256:(half + 1) * 256],
                                hpT_sb[:, hc * P + b_idx:hc * P + b_idx + 1],
                            )
                        tanh_t = tanh_pool.tile([128, 512], BF16)
                        nc.scalar.activation(tanh_t[:], parg_t[:], AF.Tanh)
                        pending.append((e_ps, hc, tanh_t, j))
                        if len(pending) >= 8:
                            flush_e()
                while pending:
                    flush_e()

                # bd half h only has nonzero columns for its own 32 b-rows, so
                # each ctx half-accumulation yields a disjoint row range: no
                # sum needed, and each half's transpose chain starts as soon
                # as its own matmuls finish (hiding under the other half / z)
                ctp = wpool.tile([P, H], BF16)

                def ctx_chain(hi2):
                    ctx_half(hi2)
                    rs2 = slice(32 * hi2, 32 * (hi2 + 1))
                    nc.vector.tensor_copy(ctx_bf[rs2, :], ctx_haps[hi2][rs2, :])
                    nc.vector.transpose(ctp[rs2, :], ctx_bf[rs2, :])
                    # merged block scatter: per partition-base, one 3D-AP copy
                    # moves all four kk blocks (16 copies -> 4)
                    src3 = ctp[rs2, :].rearrange(
                        "p (kk vm lb) -> p kk vm lb", kk=4, vm=4)
                    for vm in range(4):
                        pb = vm * 32
                        dst3 = xs_ctx[pb:pb + 32, :].rearrange(
                            "p (kk m) -> p kk m", kk=4)
                        nc.vector.tensor_copy(
                            dst3[:, :, 32 * hi2:32 * hi2 + 32],
                            src3[:, :, vm, :],
                        )

                softmax_half(1)
                for kk in range(4):
                    issue_kr(kk)
                # ctx half 0 runs inside the psB/psE scope (PSUM 8/8 in use)
                # so the pool-exit barrier hides under it instead of holing
                # the PE between B and ctx
                ctx_chain(0)
              if True:
                ctx_chain(1)

            # ---- phase E: LSTM cell ----
            # xstack split: the context-independent rows live in their own
            # tensor so the first z-matmuls don't wait on the ctx transpose
            xs_rest = wpool.tile([128, 5 * P], BF16)
            nc.sync.dma_start(xs_rest[:], xrt[:])

            gi = wpool.tile([P, H], F32)
            gf = wpool.tile([P, H], F32)
            gg = wpool.tile([P, H], F32)
            go = wpool.tile([P, H], F32)
            with tc.tile_pool(name="psZ", bufs=1, space="PSUM") as psZ:
                z_ps = psZ.tile([P, 4 * H], F32)
                # context-independent rows (onehots/prev_h/bias) first, so the
                # PE works while the context transpose chain completes
                for kk in list(range(4, 9)) + list(range(4)):
                    kr_t = kr_tiles[kk]
                    xsl = (xs_ctx[:, kk * P:(kk + 1) * P] if kk < 4 else
                           xs_rest[:, (kk - 4) * P:(kk - 3) * P])
                    for nn in range(4):
                        nc.tensor.matmul(
                            z_ps[:, nn * 512:(nn + 1) * 512],
                            xsl,
                            kr_t[:, nn * 512:(nn + 1) * 512],
                            start=(kk == 4), stop=(kk == 3),
                        )
                nc.scalar.activation(gi[:], z_ps[:, 0:512], AF.Sigmoid)
                nc.scalar.activation(gf[:], z_ps[:, 512:1024], AF.Sigmoid)
                nc.scalar.activation(gg[:], z_ps[:, 1024:1536], AF.Tanh)
                nc.scalar.activation(go[:], z_ps[:, 1536:2048], AF.Sigmoid)

            pc_sb = wpool.tile([P, H], F32)
            nc.sync.dma_start(pc_sb[:], pc[:])
            t1 = wpool.tile([P, H], F32)
            t2 = wpool.tile([P, H], F32)
            c_sb = wpool.tile([P, H], F32)
            nc.vector.tensor_mul(t1[:], gf[:], pc_sb[:])
            nc.vector.tensor_mul(t2[:], gi[:], gg[:])
            nc.vector.tensor_add(c_sb[:], t1[:], t2[:])
            nc.sync.dma_start(c_out[:], c_sb[:])
            tc_sb = wpool.tile([P, H], F32)
            nc.scalar.activation(tc_sb[:], c_sb[:], AF.Tanh)
            h_sb = wpool.tile([P, H], F32)
            nc.vector.tensor_mul(h_sb[:], go[:], tc_sb[:])
            nc.sync.dma_start(h_out[:], h_sb[:])

    nc.compile()
    return nc


def _get_nc():
    global _CACHED
    if _CACHED is None:
        _CACHED = _build()
    return _CACHED


def _prep_in_maps(inputs):
    batch_H = np.asarray(inputs["batch_H"], np.float32)
    prev_h = np.asarray(inputs["prev_h"], np.float32)
    prev_c = np.asarray(inputs["prev_c"], np.float32)
    onehots = np.asarray(inputs["char_onehots"], np.float32)
    Wi = np.asarray(inputs["Wi"], np.float32)
    Wh = np.asarray(inputs["Wh"], np.float32)
    bh = np.asarray(inputs["bh"], np.float32)
    Ws = np.asarray(inputs["Ws"], np.float32)
    K = np.asarray(inputs["K"], np.float32)
    R = np.asarray(inputs["R"], np.float32)
    b = np.asarray(inputs["b"], np.float32)

    def pack(a, k):  # [(k p), n] -> [p, (k n)]
        n = a.shape[1]
        return np.ascontiguousarray(
            a.reshape(k, 128, n).transpose(1, 0, 2).reshape(128, k * n))

    wi_b = pack(Wi, 4).astype(bf16)
    ws_cols = np.ascontiguousarray(Ws.reshape(4, 128).T)      # [p, kc]
    ws_b = np.repeat(ws_cols[:, :, None], 128, axis=2).reshape(128, 512).astype(bf16)
    whE = np.zeros((640, H), np.float32)
    whE[:H] = Wh
    whE[H] = bh
    whE_b = pack(whE, 5).astype(bf16)
    krE = np.zeros((KXP, 4 * H), np.float32)
    krE[:C + NCLS] = K
    krE[C + NCLS:C + NCLS + H] = R
    krE[C + NCLS + H] = b
    krE_b = krE.astype(bf16)

    in_maps = []
    for i in range(NCORES):
        sl = slice(i * P, (i + 1) * P)
        bh_s = batch_H[sl].reshape(BT, C)
        ph_s = prev_h[sl]                       # [64, 512]
        phTe = np.zeros((640, P), np.float32)
        phTe[:H] = ph_s.T
        phTe[H] = 1.0
        xrt = np.zeros((640, P), np.float32)
        xrt[:NCLS] = onehots[sl].T
        xrt[NCLS:NCLS + H] = ph_s.T
        xrt[NCLS + H] = 1.0
        bhn_p = np.ascontiguousarray(
            bh_s.reshape(32, 4, 128, C).transpose(0, 2, 1, 3).reshape(BT // 4, 4 * C))
        in_maps.append({
            "bhT": np.ascontiguousarray(bh_s.T).astype(bf16),
            "bhN": bhn_p.astype(bf16),
            "wi": wi_b,
            "whE": whE_b,
            "phTe": pack(phTe, 5).astype(bf16),
            "ws": ws_b,
            "xrt": pack(xrt, 5).astype(bf16),
            "krE": krE_b,
            "pc": prev_c[sl],
        })
    return in_maps


def run(inputs, **run_kwargs):
    nc = _get_nc()
    in_maps = _prep_in_maps(inputs)
    res = run_bass_kernel_spmd(nc, in_maps, core_ids=list(range(NCORES)),
                               **run_kwargs)
    h = np.concatenate([res.results[i]["h_out"] for i in range(NCORES)], 0)
    c = np.concatenate([res.results[i]["c_out"] for i in range(NCORES)], 0)
    alpha = np.concatenate([res.results[i]["alpha_out"] for i in range(NCORES)], 0)
    return (h, c, alpha.reshape(B, T, 1)), res


def kernel(**inputs):
    (h, c, alpha), _ = run(inputs)
    return (h, c, alpha)
